# revision 17
# baseline (speedup 1.0000x reference)
"""Trainium2 Bass kernel for CustomTaylorLayer.

Computes out[b, j] = sum_{i,k} coef[j, i, k] * tanh(x[b, i] * r)^k
for x:[8192,1024], coef:[1024,1024,8], r scalar.

Strategy: data-parallel over the batch across 8 NeuronCores (1024 rows
per core). The k=0 term is an exact host-side column sum added after the
gather. k=1 runs as bf16 matmuls on b1=tanh. k=2..7 run as fp8(e4m3)
DoubleRow matmuls (2 weights/PE cell, 256-wide contraction per issue) on
variance-reduced channels

  g2 = 2(t2 - mu2),          g3 = t(t2-a3),      g4 = (t2-a3)(t2-a4),
  g5 = 2*g3*(t2-a5),         g6 = 2*g4*(t2-a6),  g7 = g5*(t2-a7)

(t2 = tanh^2; factored orthogonal-ish polynomials whose rms is 3-10x
smaller than the raw powers, so fp8's 3.6% relative rounding hits much
smaller values; the 2x scales keep the channels out of fp8 subnormals
and divide out of the weights). The host exactly re-expresses the power
basis in these channels (triangular polynomial solve) and folds the
transform into the weights; constants fold into the k=0 bias.

Channels are built with bf16 DVE ops (fast 16-bit path, one mul per
channel) and cast to fp8 on the Act engine in k-major order matched to
the PE's PSUM-accumulation rounds. All 7 k-terms of an output tile
accumulate into a single PSUM bank (8 banks = 8 j-tiles in flight per
batch-half); DVE flushes each bank once to SBUF. Input DMAs are spread
over the sync/vector/gpsimd rings so x and the k-ordered weights land
just ahead of their consumers.
"""

import numpy as np
import ml_dtypes
from contextlib import ExitStack

B, IN, OUT, K = 8192, 1024, 1024, 8
NCORES = 8
BLOC = B // NCORES          # 1024 batch rows per core
NI = IN // 128              # 8 i-tiles
NJ = OUT // 128              # 8 j-tiles
NH = BLOC // 512            # 2 batch halves (PSUM bank = 512 fp32)

# Sequential weighted-least-squares roots of the factored channels under
# the empirical distribution of t = tanh(x), x ~ N(0,1), and E[t^2].
A3 = 0.641655
A4 = 0.153814
A5 = 0.500317
A6 = 0.705566
A7 = 0.570340
MU2 = 0.39426075880007483
# fp8 cast scales (applied on device, divided out of W on the host).
SC = {2: 2.0, 3: 1.0, 4: 1.0, 5: 2.0, 6: 2.0, 7: 2.0}

_NC_CACHE = {}


def _build_nc():
    import concourse.bacc as bacc
    import concourse.mybir as mybir
    import concourse.tile as tile

    dt = mybir.dt
    AF = mybir.ActivationFunctionType
    ALU = mybir.AluOpType
    DR = mybir.MatmulPerfMode.DoubleRow
    f32 = dt.float32
    bf16 = dt.bfloat16
    f8 = dt.float8e4

    nc = bacc.Bacc("TRN2", target_bir_lowering=False, debug=False)

    xt_d = nc.dram_tensor("xt", [IN, BLOC], bf16, kind="ExternalInput").ap()
    w1_d = nc.dram_tensor("w1", [128, NI, OUT], bf16, kind="ExternalInput").ap()
    wg_d = {k: nc.dram_tensor(f"wg{k}", [128, NI, OUT], f8,
                              kind="ExternalInput").ap() for k in range(2, K)}
    rng_d = nc.dram_tensor("rng", [1, 1], f32, kind="ExternalInput").ap()
    out_d = nc.dram_tensor("outT", [OUT, BLOC], f32, kind="ExternalOutput").ap()

    with tile.TileContext(nc) as tc, ExitStack() as ctx:
        sb = ctx.enter_context(tc.tile_pool(name="sb", bufs=1))
        xp = ctx.enter_context(tc.tile_pool(name="xp", bufs=3))
        sp = ctx.enter_context(tc.tile_pool(name="sp", bufs=2))
        wp = ctx.enter_context(tc.tile_pool(name="wp", bufs=9))
        op = ctx.enter_context(tc.tile_pool(name="op", bufs=4))
        pp = ctx.enter_context(tc.tile_pool(name="pp", bufs=8, space="PSUM"))

        # xs for h=1 as one persistent tile: a single 2MB DMA on the
        # vector ring overlaps the fine-grained h=0 stream on sync.
        xsh1 = sb.tile([128, NI, 512], bf16, tag="xsh1")
        r_col = sb.tile([128, 1], f32, tag="rcol")
        nc.sync.dma_start(r_col[:], rng_d.to_broadcast((128, 1)))

        # Persistent channel tensors, [128, i-tile, batch]
        b1 = sb.tile([128, NI, BLOC], bf16, tag="b1")
        g = {k: sb.tile([128, NI, BLOC], f8, tag=f"g{k}", name=f"g{k}")
             for k in range(2, K)}
        # Persistent weights
        w1s = sb.tile([128, NI, OUT], bf16, tag="w1s")
        wgs = {k: sb.tile([128, NI, OUT], f8, tag=f"wg{k}s", name=f"wg{k}s")
               for k in range(2, K)}

        ones = sb.tile([128, 512], bf16, tag="ones")
        nc.vector.memset(ones[:], 1.0)
        onesf = sb.tile([128, 1], f32, tag="onesf")
        nc.vector.memset(onesf[:], 1.0)

        # Preload the ACT tanh table before any real data arrives.
        warm = sb.tile([128, 1], f32, tag="warm")
        nc.scalar.activation(warm[:], onesf[:], AF.Tanh)

        # DMA plan. The three dynamic rings share the 16 DMA engines
        # (~400GB/s aggregate, saturated during startup), so the GLOBAL
        # trigger order must follow the PE's need order: x-h0/w1 first,
        # then wg2 (PE rounds run k1a, k2, k1b, k3..k7), then the rest.
        # gpsimd ring: w1 early half, then wg3, wg4, wg7.
        for ii in range(4):
            nc.gpsimd.dma_start(w1s[:, ii, :], w1_d[:, ii, :])
        nc.gpsimd.dma_start(wgs[3][:], wg_d[3][:])
        nc.gpsimd.dma_start(wgs[4][:], wg_d[4][:])
        nc.gpsimd.dma_start(wgs[7][:], wg_d[7][:])

        # Tiny warmup matmuls ramp the PE clock without delaying k=1.
        wps = pp.tile([128, 512], f32, tag="ps")
        for wv in range(12):
            nc.tensor.matmul(wps[:, 0:128], ones[:, 0:128], ones[:, 0:128],
                             start=(wv == 0), stop=(wv == 11))

        # sync ring: even h=0 x chunks first (x has global priority:
        # everything depends on it), then w1 late half, xsh1, wg6.
        xss = {}
        for it in range(NI):
            xss[it] = xp.tile([128, 512], bf16, tag="xs", name=f"xs{it}")
        for it in range(0, NI, 2):
            nc.sync.dma_start(xss[it][:],
                              xt_d[it * 128:(it + 1) * 128, 0:512])
        for ii in range(4, NI):
            nc.sync.dma_start(w1s[:, ii, :], w1_d[:, ii, :])
        nc.sync.dma_start(xsh1[:], xt_d[:, 512:1024].rearrange(
            "(i p) b -> p i b", p=128))
        nc.sync.dma_start(wgs[6][:], wg_d[6][:])

        # Act queue: odd x chunks, then wg2/wg5 interleaved with tanh.
        for it in range(1, NI, 2):
            nc.scalar.dma_start(xss[it][:],
                                xt_d[it * 128:(it + 1) * 128, 0:512])
        nc.scalar.dma_start(wgs[2][:, 0:4, :], wg_d[2][:, 0:4, :])
        nc.scalar.dma_start(wgs[2][:, 4:NI, :], wg_d[2][:, 4:NI, :])
        for it in range(NI):
            nc.scalar.activation(b1[:, it, 0:512], xss[it][:], AF.Tanh,
                                 scale=r_col[:, 0:1])
            if it == 4:
                nc.scalar.dma_start(wgs[5][:], wg_d[5][:])

        b2s = {}

        def emit_dve_a(h):
            # pass A, level-major: all b2 muls first (g2 casts unblock
            # right behind the tanh stream), then v3/w3, then v4/w4.
            hs = slice(h * 512, (h + 1) * 512)
            tiles = []
            for it in range(NI):
                b2 = wp.tile([128, 512], bf16, tag="b2", name=f"b2_{h}_{it}")
                v3 = wp.tile([128, 512], bf16, tag="v3", name=f"v3_{h}_{it}")
                v4 = sp.tile([128, 512], bf16, tag="v4", name=f"v4_{h}_{it}",
                             bufs=4)
                w3 = wp.tile([128, 512], bf16, tag="w3", name=f"w3_{h}_{it}")
                w4 = wp.tile([128, 512], bf16, tag="w4", name=f"w4_{h}_{it}")
                b2s[(h, it)] = b2
                tiles.append((v3, v4, w3, w4))
            for it in range(NI):
                nc.vector.tensor_mul(b2s[(h, it)][:], b1[:, it, hs],
                                     b1[:, it, hs])
            for it in range(NI):
                v3, v4, w3, w4 = tiles[it]
                nc.vector.tensor_scalar_add(v3[:], b2s[(h, it)][:],
                                            float(-A3))
                nc.vector.tensor_mul(w3[:], v3[:], b1[:, it, hs])
            for it in range(NI):
                v3, v4, w3, w4 = tiles[it]
                nc.vector.tensor_scalar_add(v4[:], b2s[(h, it)][:],
                                            float(-A4))
                nc.vector.tensor_mul(w4[:], v3[:], v4[:])
            return [(w3, w4) for (v3, v4, w3, w4) in tiles]

        def emit_dve_b(h, ab):
            # pass B: w5 = 2*w3*(t2-a5), w6 = 2*w4*(t2-a6), w7 = w5*(t2-a7)
            out = []
            for it in range(NI):
                w3, w4 = ab[it]
                b2 = b2s[(h, it)]
                v5 = sp.tile([128, 512], bf16, tag="v5")
                v6 = sp.tile([128, 512], bf16, tag="v6")
                v7 = sp.tile([128, 512], bf16, tag="v7")
                w5 = sp.tile([128, 512], bf16, tag="w5")
                w6 = sp.tile([128, 512], bf16, tag="w6")
                w7 = sp.tile([128, 512], bf16, tag="w7")
                out.append((w5, w6, w7))
                nc.vector.tensor_scalar(v5[:], b2[:], float(-A5), 2.0,
                                        op0=ALU.add, op1=ALU.mult)
                nc.vector.tensor_mul(w5[:], w3[:], v5[:])
                nc.vector.tensor_scalar(v6[:], b2[:], float(-A6), 2.0,
                                        op0=ALU.add, op1=ALU.mult)
                nc.vector.tensor_mul(w6[:], w4[:], v6[:])
                nc.vector.tensor_scalar_add(v7[:], b2[:], float(-A7))
                nc.vector.tensor_mul(w7[:], w5[:], v7[:])
            return out

        def emit_casts_a(h, ab):
            # k-major so each PE round's channel completes as early as
            # possible; g5..g7 casts are deferred to emit_casts_b.
            hs = slice(h * 512, (h + 1) * 512)
            for it in range(NI):
                nc.scalar.activation(g[2][:, it, hs], b2s[(h, it)][:],
                                     AF.Copy, scale=float(SC[2]),
                                     bias=float(-SC[2] * MU2))
            for it in range(NI):
                nc.scalar.activation(g[3][:, it, hs], ab[it][0][:], AF.Copy)
            for it in range(NI):
                nc.scalar.activation(g[4][:, it, hs], ab[it][1][:], AF.Copy)

        def emit_casts_b(h, wb):
            hs = slice(h * 512, (h + 1) * 512)
            for m in range(3):
                for it in range(NI):
                    nc.scalar.activation(g[5 + m][:, it, hs], wb[it][m][:],
                                         AF.Copy)

        def emit_pe(h, pss):
            hs = slice(h * 512, (h + 1) * 512)

            def k1_half(iis):
                for ii in iis:
                    for j in range(NJ):
                        nc.tensor.matmul(
                            pss[j][:], w1s[:, ii, j * 128:(j + 1) * 128],
                            b1[:, ii, hs], start=(ii == 0), stop=False)

            def dr_round(k):
                for ip in range(NI // 2):
                    for j in range(NJ):
                        nc.tensor.matmul(
                            pss[j][:],
                            wgs[k][:, 2 * ip:2 * ip + 2,
                                   j * 128:(j + 1) * 128],
                            g[k][:, 2 * ip:2 * ip + 2, hs],
                            start=False,
                            stop=(k == K - 1 and ip == NI // 2 - 1),
                            perf_mode=DR)

            # h=0: interleave the k1 halves with k2 so the PE never
            # outruns the w1/wg2 DMA streams; h=1: natural order.
            if h == 0:
                k1_half(range(4))
                dr_round(2)
                k1_half(range(4, NI))
            else:
                k1_half(range(NI))
                dr_round(2)
            for k in range(3, K):
                dr_round(k)

        def emit_flush(h, pss, engines):
            hs = slice(h * 512, (h + 1) * 512)
            for j in range(NJ):
                ot = op.tile([128, 512], f32, tag="ot")
                nc.vector.tensor_copy(ot[:], pss[j][:])
                engines[j % len(engines)].dma_start(
                    out_d[j * 128:(j + 1) * 128, hs], ot[:])

        pss = {h: [pp.tile([128, 512], f32, tag="ps", name=f"ps{h}_{j}")
                   for j in range(NJ)] for h in range(NH)}

        ab0 = emit_dve_a(0)
        emit_casts_a(0, ab0)
        wb0 = emit_dve_b(0, ab0)
        # h=1 tanh sits between the h0-A and h0-B cast groups: its xs
        # lands by ~17us and nothing on the PE needs it before ~70us.
        for it in range(NI):
            nc.scalar.activation(b1[:, it, 512:1024], xsh1[:, it, :],
                                 AF.Tanh, scale=r_col[:, 0:1])
        emit_casts_b(0, wb0)
        emit_pe(0, pss[0])
        ab1 = emit_dve_a(1)
        emit_casts_a(1, ab1)
        wb1 = emit_dve_b(1, ab1)
        emit_casts_b(1, wb1)
        emit_flush(0, pss[0], [nc.gpsimd])
        emit_pe(1, pss[1])
        emit_flush(1, pss[1], [nc.gpsimd, nc.sync])

    nc.compile()
    return nc


def _get_nc():
    if "nc" not in _NC_CACHE:
        _NC_CACHE["nc"] = _build_nc()
    return _NC_CACHE["nc"]


def _channel_polys():
    """Power-basis coefficients of the 7 channels, and the inverse map."""
    import numpy.polynomial.polynomial as P

    def pm(*ps):
        r = np.array([1.0])
        for p in ps:
            r = P.polymul(r, p)
        return r

    q = {k: np.array([-a, 0.0, 1.0]) for k, a in
         ((3, A3), (4, A4), (5, A5), (6, A6), (7, A7))}
    t = np.array([0.0, 1.0])
    CH = {1: t, 2: np.array([0.0, 0.0, 1.0]),
          3: pm(t, q[3]), 4: pm(q[3], q[4]), 5: pm(t, q[3], q[5]),
          6: pm(q[3], q[4], q[6]), 7: pm(t, q[3], q[5], q[7])}
    C = np.zeros((7, 8))
    for m in range(1, 8):
        cc = CH[m]
        C[m - 1, :len(cc)] = cc
    M = C[:, 1:8]                       # channel_m = consts + M @ powers
    consts = C[:, 0]
    Binv = np.linalg.inv(M)             # powers = Binv @ (channels - consts)
    return Binv, consts


def _pack_w(w, dtype):
    # [OUT, IN] -> [128, NI, OUT]
    a = np.ascontiguousarray(w.T.reshape(NI, 128, OUT).transpose(1, 0, 2))
    return a.astype(dtype)


def _make_in_maps(x, tanh_range, coef):
    x = np.asarray(x, dtype=np.float32)
    coef = np.asarray(coef, dtype=np.float64)
    W = {k: coef[:, :, k] for k in range(K)}
    Binv, consts = _channel_polys()
    Wp = {m: sum(W[k] * Binv[k - 1, m - 1] for k in range(1, 8))
          for m in range(1, 8)}
    s = W[0].sum(axis=1)
    for k in range(1, 8):
        cst = sum(Binv[k - 1, m - 1] * consts[m - 1] for m in range(1, 8))
        s -= cst * W[k].sum(axis=1)
    # channel 2 is fed as SC2*(t2 - MU2): fold the mean term into s.
    s += MU2 * Wp[2].sum(axis=1)
    bf = ml_dtypes.bfloat16
    f8 = ml_dtypes.float8_e4m3
    shared = {"w1": _pack_w(Wp[1], bf),
              "rng": np.asarray(tanh_range, np.float32).reshape(1, 1)}
    for k in range(2, K):
        shared[f"wg{k}"] = _pack_w(Wp[k] / SC[k], f8)
    in_maps = []
    xbf = x.astype(ml_dtypes.bfloat16)
    for c in range(NCORES):
        xt = np.ascontiguousarray(xbf[c * BLOC:(c + 1) * BLOC, :].T)
        in_maps.append({"xt": xt, **shared})
    return in_maps, s.astype(np.float32)


def _ensure_ntff_hook():
    """Register the axon NTFF profile hook if the image's antenv lacks it."""
    import sys
    import types
    try:
        from antenv.axon_hooks import get_axon_ntff_profile_hook  # noqa: F401
        return
    except ImportError:
        pass
    try:
        from trn_agent_boot.trn_boot import _ntff_profile_via_ctypes
        hook = _ntff_profile_via_ctypes("/opt/axon/libaxon_pjrt.so")
    except Exception:
        hook = None
    mod = types.ModuleType("antenv.axon_hooks")
    state = {"hook": hook}
    mod.set_axon_ntff_profile_hook = lambda h: state.__setitem__("hook", h)
    mod.get_axon_ntff_profile_hook = lambda: state["hook"]
    sys.modules["antenv.axon_hooks"] = mod
    import antenv
    antenv.axon_hooks = mod


def _run(x, tanh_range, coef, trace=False):
    from concourse.bass_utils import run_bass_kernel_spmd

    if trace:
        _ensure_ntff_hook()

    nc = _get_nc()
    in_maps, s = _make_in_maps(x, tanh_range, coef)
    res = run_bass_kernel_spmd(nc, in_maps, core_ids=list(range(NCORES)),
                               trace=trace)
    out = np.empty((B, OUT), dtype=np.float32)
    for c in range(NCORES):
        out[c * BLOC:(c + 1) * BLOC, :] = res.results[c]["outT"].T
    out += s[None, :]
    return out, res


def kernel(x, tanh_range, coef):
    out, _ = _run(x, tanh_range, coef, trace=False)
    return out


# revision 18
# speedup vs baseline: 1.0080x; 1.0080x over previous
"""Trainium2 Bass kernel for CustomTaylorLayer.

Computes out[b, j] = sum_{i,k} coef[j, i, k] * tanh(x[b, i] * r)^k
for x:[8192,1024], coef:[1024,1024,8], r scalar.

Strategy: data-parallel over the batch across 8 NeuronCores (1024 rows
per core). The k=0 term is an exact host-side column sum added after the
gather. k=1 runs as bf16 matmuls on b1=tanh. k=2..7 run as fp8(e4m3)
DoubleRow matmuls (2 weights/PE cell, 256-wide contraction per issue) on
variance-reduced channels

  g2 = 2(t2 - mu2),          g3 = t(t2-a3),      g4 = (t2-a3)(t2-a4),
  g5 = 2*g3*(t2-a5),         g6 = 2*g4*(t2-a6),  g7 = g5*(t2-a7)

(t2 = tanh^2; factored orthogonal-ish polynomials whose rms is 3-10x
smaller than the raw powers, so fp8's 3.6% relative rounding hits much
smaller values; the 2x scales keep the channels out of fp8 subnormals
and divide out of the weights). The host exactly re-expresses the power
basis in these channels (triangular polynomial solve) and folds the
transform into the weights; constants fold into the k=0 bias.

Channels are built with bf16 DVE ops (fast 16-bit path, one mul per
channel) and cast to fp8 on the Act engine in k-major order matched to
the PE's PSUM-accumulation rounds. All 7 k-terms of an output tile
accumulate into a single PSUM bank (8 banks = 8 j-tiles in flight per
batch-half); DVE flushes each bank once to SBUF. Input DMAs are spread
over the sync/vector/gpsimd rings so x and the k-ordered weights land
just ahead of their consumers.
"""

import numpy as np
import ml_dtypes
from contextlib import ExitStack

B, IN, OUT, K = 8192, 1024, 1024, 8
NCORES = 8
BLOC = B // NCORES          # 1024 batch rows per core
NI = IN // 128              # 8 i-tiles
NJ = OUT // 128              # 8 j-tiles
NH = BLOC // 512            # 2 batch halves (PSUM bank = 512 fp32)

# Sequential weighted-least-squares roots of the factored channels under
# the empirical distribution of t = tanh(x), x ~ N(0,1), and E[t^2].
A3 = 0.641655
A4 = 0.153814
A5 = 0.500317
A6 = 0.705566
A7 = 0.570340
MU2 = 0.39426075880007483
# fp8 cast scales (applied on device, divided out of W on the host).
SC = {2: 2.0, 3: 1.0, 4: 1.0, 5: 2.0, 6: 2.0, 7: 2.0}

_NC_CACHE = {}


def _build_nc():
    import concourse.bacc as bacc
    import concourse.mybir as mybir
    import concourse.tile as tile

    dt = mybir.dt
    AF = mybir.ActivationFunctionType
    ALU = mybir.AluOpType
    DR = mybir.MatmulPerfMode.DoubleRow
    f32 = dt.float32
    bf16 = dt.bfloat16
    f8 = dt.float8e4

    nc = bacc.Bacc("TRN2", target_bir_lowering=False, debug=False)

    xt_d = nc.dram_tensor("xt", [IN, BLOC], bf16, kind="ExternalInput").ap()
    w1_d = nc.dram_tensor("w1", [128, NI, OUT], bf16, kind="ExternalInput").ap()
    wg_d = {k: nc.dram_tensor(f"wg{k}", [128, NI, OUT], f8,
                              kind="ExternalInput").ap() for k in range(2, K)}
    rng_d = nc.dram_tensor("rng", [1, 1], f32, kind="ExternalInput").ap()
    out_d = nc.dram_tensor("outT", [OUT, BLOC], f32, kind="ExternalOutput").ap()

    with tile.TileContext(nc) as tc, ExitStack() as ctx:
        sb = ctx.enter_context(tc.tile_pool(name="sb", bufs=1))
        xp = ctx.enter_context(tc.tile_pool(name="xp", bufs=3))
        sp = ctx.enter_context(tc.tile_pool(name="sp", bufs=2))
        wp = ctx.enter_context(tc.tile_pool(name="wp", bufs=9))
        op = ctx.enter_context(tc.tile_pool(name="op", bufs=4))
        pp = ctx.enter_context(tc.tile_pool(name="pp", bufs=8, space="PSUM"))

        # xs for h=1 as one persistent tile: a single 2MB DMA on the
        # vector ring overlaps the fine-grained h=0 stream on sync.
        xsh1 = sb.tile([128, NI, 512], bf16, tag="xsh1")
        r_col = sb.tile([128, 1], f32, tag="rcol")
        nc.sync.dma_start(r_col[:], rng_d.to_broadcast((128, 1)))

        # Persistent channel tensors, [128, i-tile, batch]
        b1 = sb.tile([128, NI, BLOC], bf16, tag="b1")
        g = {k: sb.tile([128, NI, BLOC], f8, tag=f"g{k}", name=f"g{k}")
             for k in range(2, K)}
        # Persistent weights
        w1s = sb.tile([128, NI, OUT], bf16, tag="w1s")
        wgs = {k: sb.tile([128, NI, OUT], f8, tag=f"wg{k}s", name=f"wg{k}s")
               for k in range(2, K)}

        ones = sb.tile([128, 512], bf16, tag="ones")
        nc.vector.memset(ones[:], 1.0)
        onesf = sb.tile([128, 1], f32, tag="onesf")
        nc.vector.memset(onesf[:], 1.0)

        # Preload the ACT tanh table before any real data arrives.
        warm = sb.tile([128, 1], f32, tag="warm")
        nc.scalar.activation(warm[:], onesf[:], AF.Tanh)

        # DMA plan. The three dynamic rings share the 16 DMA engines
        # (~400GB/s aggregate, saturated during startup), so the GLOBAL
        # trigger order must follow the PE's need order: x-h0/w1 first,
        # then wg2 (PE rounds run k1a, k2, k1b, k3..k7), then the rest.
        # gpsimd ring: all of w1 in ii order, then wg5, wg7.
        for ii in range(NI):
            nc.gpsimd.dma_start(w1s[:, ii, :], w1_d[:, ii, :])
        nc.gpsimd.dma_start(wgs[5][:], wg_d[5][:])
        nc.gpsimd.dma_start(wgs[7][:], wg_d[7][:])

        # Tiny warmup matmuls ramp the PE clock without delaying k=1.
        wps = pp.tile([128, 512], f32, tag="ps")
        for wv in range(12):
            nc.tensor.matmul(wps[:, 0:128], ones[:, 0:128], ones[:, 0:128],
                             start=(wv == 0), stop=(wv == 11))

        # sync ring: even h=0 x chunks first (x has global priority:
        # everything depends on it), then w1 late half, xsh1, wg6.
        xss = {}
        for it in range(NI):
            xss[it] = xp.tile([128, 512], bf16, tag="xs", name=f"xs{it}")
        for it in range(0, NI, 2):
            nc.sync.dma_start(xss[it][:],
                              xt_d[it * 128:(it + 1) * 128, 0:512])
        nc.sync.dma_start(wgs[3][:], wg_d[3][:])
        nc.sync.dma_start(wgs[4][:], wg_d[4][:])
        nc.sync.dma_start(xsh1[:], xt_d[:, 512:1024].rearrange(
            "(i p) b -> p i b", p=128))
        nc.sync.dma_start(wgs[6][:], wg_d[6][:])

        # Act queue: odd x chunks, then wg2/wg5 interleaved with tanh.
        for it in range(1, NI, 2):
            nc.scalar.dma_start(xss[it][:],
                                xt_d[it * 128:(it + 1) * 128, 0:512])
        nc.scalar.dma_start(wgs[2][:, 0:4, :], wg_d[2][:, 0:4, :])
        nc.scalar.dma_start(wgs[2][:, 4:NI, :], wg_d[2][:, 4:NI, :])
        for it in range(NI):
            nc.scalar.activation(b1[:, it, 0:512], xss[it][:], AF.Tanh,
                                 scale=r_col[:, 0:1])


        b2s = {}

        def emit_dve_a(h):
            # pass A, level-major: all b2 muls first (g2 casts unblock
            # right behind the tanh stream), then v3/w3, then v4/w4.
            hs = slice(h * 512, (h + 1) * 512)
            tiles = []
            for it in range(NI):
                b2 = wp.tile([128, 512], bf16, tag="b2", name=f"b2_{h}_{it}")
                v3 = wp.tile([128, 512], bf16, tag="v3", name=f"v3_{h}_{it}")
                v4 = sp.tile([128, 512], bf16, tag="v4", name=f"v4_{h}_{it}",
                             bufs=4)
                w3 = wp.tile([128, 512], bf16, tag="w3", name=f"w3_{h}_{it}")
                w4 = wp.tile([128, 512], bf16, tag="w4", name=f"w4_{h}_{it}")
                b2s[(h, it)] = b2
                tiles.append((v3, v4, w3, w4))
            for it in range(NI):
                nc.vector.tensor_mul(b2s[(h, it)][:], b1[:, it, hs],
                                     b1[:, it, hs])
            for it in range(NI):
                v3, v4, w3, w4 = tiles[it]
                nc.vector.tensor_scalar_add(v3[:], b2s[(h, it)][:],
                                            float(-A3))
                nc.vector.tensor_mul(w3[:], v3[:], b1[:, it, hs])
            for it in range(NI):
                v3, v4, w3, w4 = tiles[it]
                nc.vector.tensor_scalar_add(v4[:], b2s[(h, it)][:],
                                            float(-A4))
                nc.vector.tensor_mul(w4[:], v3[:], v4[:])
            return [(w3, w4) for (v3, v4, w3, w4) in tiles]

        def emit_dve_b(h, ab):
            # pass B: w5 = 2*w3*(t2-a5), w6 = 2*w4*(t2-a6), w7 = w5*(t2-a7)
            out = []
            for it in range(NI):
                w3, w4 = ab[it]
                b2 = b2s[(h, it)]
                v5 = sp.tile([128, 512], bf16, tag="v5")
                v6 = sp.tile([128, 512], bf16, tag="v6")
                v7 = sp.tile([128, 512], bf16, tag="v7")
                w5 = sp.tile([128, 512], bf16, tag="w5")
                w6 = sp.tile([128, 512], bf16, tag="w6")
                w7 = sp.tile([128, 512], bf16, tag="w7")
                out.append((w5, w6, w7))
                nc.vector.tensor_scalar(v5[:], b2[:], float(-A5), 2.0,
                                        op0=ALU.add, op1=ALU.mult)
                nc.vector.tensor_mul(w5[:], w3[:], v5[:])
                nc.vector.tensor_scalar(v6[:], b2[:], float(-A6), 2.0,
                                        op0=ALU.add, op1=ALU.mult)
                nc.vector.tensor_mul(w6[:], w4[:], v6[:])
                nc.vector.tensor_scalar_add(v7[:], b2[:], float(-A7))
                nc.vector.tensor_mul(w7[:], w5[:], v7[:])
            return out

        def emit_casts_a(h, ab):
            # k-major so each PE round's channel completes as early as
            # possible; g5..g7 casts are deferred to emit_casts_b.
            hs = slice(h * 512, (h + 1) * 512)
            for it in range(NI):
                nc.scalar.activation(g[2][:, it, hs], b2s[(h, it)][:],
                                     AF.Copy, scale=float(SC[2]),
                                     bias=float(-SC[2] * MU2))
            for it in range(NI):
                nc.scalar.activation(g[3][:, it, hs], ab[it][0][:], AF.Copy)
            for it in range(NI):
                nc.scalar.activation(g[4][:, it, hs], ab[it][1][:], AF.Copy)

        def emit_casts_b(h, wb):
            hs = slice(h * 512, (h + 1) * 512)
            for m in range(3):
                for it in range(NI):
                    nc.scalar.activation(g[5 + m][:, it, hs], wb[it][m][:],
                                         AF.Copy)

        def emit_pe(h, pss):
            hs = slice(h * 512, (h + 1) * 512)

            def k1_half(iis):
                for ii in iis:
                    for j in range(NJ):
                        nc.tensor.matmul(
                            pss[j][:], w1s[:, ii, j * 128:(j + 1) * 128],
                            b1[:, ii, hs], start=(ii == 0), stop=False)

            def dr_round(k, ips):
                for ip in ips:
                    for j in range(NJ):
                        nc.tensor.matmul(
                            pss[j][:],
                            wgs[k][:, 2 * ip:2 * ip + 2,
                                   j * 128:(j + 1) * 128],
                            g[k][:, 2 * ip:2 * ip + 2, hs],
                            start=False,
                            stop=(k == K - 1 and ip == NI // 2 - 1),
                            perf_mode=DR)

            # h=0: fine-grained interleave of k1/k2 sub-rounds so early
            # PE consumption never outruns the saturated DMA pool;
            # h=1: natural order (everything is resident).
            if h == 0:
                k1_half(range(0, 2))
                k1_half(range(2, 4))
                dr_round(2, range(2))
                k1_half(range(4, 6))
                dr_round(2, range(2, 4))
                k1_half(range(6, NI))
            else:
                k1_half(range(NI))
                dr_round(2, range(4))
            for k in range(3, K):
                dr_round(k, range(4))

        def emit_flush(h, pss, engines):
            hs = slice(h * 512, (h + 1) * 512)
            for j in range(NJ):
                ot = op.tile([128, 512], f32, tag="ot")
                nc.vector.tensor_copy(ot[:], pss[j][:])
                engines[j % len(engines)].dma_start(
                    out_d[j * 128:(j + 1) * 128, hs], ot[:])

        pss = {h: [pp.tile([128, 512], f32, tag="ps", name=f"ps{h}_{j}")
                   for j in range(NJ)] for h in range(NH)}

        ab0 = emit_dve_a(0)
        emit_casts_a(0, ab0)
        wb0 = emit_dve_b(0, ab0)
        # h=1 tanh sits between the h0-A and h0-B cast groups: its xs
        # lands by ~17us and nothing on the PE needs it before ~70us.
        for it in range(NI):
            nc.scalar.activation(b1[:, it, 512:1024], xsh1[:, it, :],
                                 AF.Tanh, scale=r_col[:, 0:1])
        emit_casts_b(0, wb0)
        emit_pe(0, pss[0])
        ab1 = emit_dve_a(1)
        emit_casts_a(1, ab1)
        wb1 = emit_dve_b(1, ab1)
        emit_casts_b(1, wb1)
        emit_flush(0, pss[0], [nc.gpsimd])
        emit_pe(1, pss[1])
        emit_flush(1, pss[1], [nc.gpsimd, nc.sync])

    nc.compile()
    return nc


def _get_nc():
    if "nc" not in _NC_CACHE:
        _NC_CACHE["nc"] = _build_nc()
    return _NC_CACHE["nc"]


def _channel_polys():
    """Power-basis coefficients of the 7 channels, and the inverse map."""
    import numpy.polynomial.polynomial as P

    def pm(*ps):
        r = np.array([1.0])
        for p in ps:
            r = P.polymul(r, p)
        return r

    q = {k: np.array([-a, 0.0, 1.0]) for k, a in
         ((3, A3), (4, A4), (5, A5), (6, A6), (7, A7))}
    t = np.array([0.0, 1.0])
    CH = {1: t, 2: np.array([0.0, 0.0, 1.0]),
          3: pm(t, q[3]), 4: pm(q[3], q[4]), 5: pm(t, q[3], q[5]),
          6: pm(q[3], q[4], q[6]), 7: pm(t, q[3], q[5], q[7])}
    C = np.zeros((7, 8))
    for m in range(1, 8):
        cc = CH[m]
        C[m - 1, :len(cc)] = cc
    M = C[:, 1:8]                       # channel_m = consts + M @ powers
    consts = C[:, 0]
    Binv = np.linalg.inv(M)             # powers = Binv @ (channels - consts)
    return Binv, consts


def _pack_w(w, dtype):
    # [OUT, IN] -> [128, NI, OUT]
    a = np.ascontiguousarray(w.T.reshape(NI, 128, OUT).transpose(1, 0, 2))
    return a.astype(dtype)


def _make_in_maps(x, tanh_range, coef):
    x = np.asarray(x, dtype=np.float32)
    coef = np.asarray(coef, dtype=np.float64)
    W = {k: coef[:, :, k] for k in range(K)}
    Binv, consts = _channel_polys()
    Wp = {m: sum(W[k] * Binv[k - 1, m - 1] for k in range(1, 8))
          for m in range(1, 8)}
    s = W[0].sum(axis=1)
    for k in range(1, 8):
        cst = sum(Binv[k - 1, m - 1] * consts[m - 1] for m in range(1, 8))
        s -= cst * W[k].sum(axis=1)
    # channel 2 is fed as SC2*(t2 - MU2): fold the mean term into s.
    s += MU2 * Wp[2].sum(axis=1)
    bf = ml_dtypes.bfloat16
    f8 = ml_dtypes.float8_e4m3
    shared = {"w1": _pack_w(Wp[1], bf),
              "rng": np.asarray(tanh_range, np.float32).reshape(1, 1)}
    for k in range(2, K):
        shared[f"wg{k}"] = _pack_w(Wp[k] / SC[k], f8)
    in_maps = []
    xbf = x.astype(ml_dtypes.bfloat16)
    for c in range(NCORES):
        xt = np.ascontiguousarray(xbf[c * BLOC:(c + 1) * BLOC, :].T)
        in_maps.append({"xt": xt, **shared})
    return in_maps, s.astype(np.float32)


def _ensure_ntff_hook():
    """Register the axon NTFF profile hook if the image's antenv lacks it."""
    import sys
    import types
    try:
        from antenv.axon_hooks import get_axon_ntff_profile_hook  # noqa: F401
        return
    except ImportError:
        pass
    try:
        from trn_agent_boot.trn_boot import _ntff_profile_via_ctypes
        hook = _ntff_profile_via_ctypes("/opt/axon/libaxon_pjrt.so")
    except Exception:
        hook = None
    mod = types.ModuleType("antenv.axon_hooks")
    state = {"hook": hook}
    mod.set_axon_ntff_profile_hook = lambda h: state.__setitem__("hook", h)
    mod.get_axon_ntff_profile_hook = lambda: state["hook"]
    sys.modules["antenv.axon_hooks"] = mod
    import antenv
    antenv.axon_hooks = mod


def _run(x, tanh_range, coef, trace=False):
    from concourse.bass_utils import run_bass_kernel_spmd

    if trace:
        _ensure_ntff_hook()

    nc = _get_nc()
    in_maps, s = _make_in_maps(x, tanh_range, coef)
    res = run_bass_kernel_spmd(nc, in_maps, core_ids=list(range(NCORES)),
                               trace=trace)
    out = np.empty((B, OUT), dtype=np.float32)
    for c in range(NCORES):
        out[c * BLOC:(c + 1) * BLOC, :] = res.results[c]["outT"].T
    out += s[None, :]
    return out, res


def kernel(x, tanh_range, coef):
    out, _ = _run(x, tanh_range, coef, trace=False)
    return out


# revision 19
# speedup vs baseline: 1.0195x; 1.0114x over previous
"""Trainium2 Bass kernel for CustomTaylorLayer.

Computes out[b, j] = sum_{i,k} coef[j, i, k] * tanh(x[b, i] * r)^k
for x:[8192,1024], coef:[1024,1024,8], r scalar.

Strategy: data-parallel over the batch across 8 NeuronCores (1024 rows
per core). The k=0 term is an exact host-side column sum added after the
gather. k=1 runs as bf16 matmuls on b1=tanh. k=2..7 run as fp8(e4m3)
DoubleRow matmuls (2 weights/PE cell, 256-wide contraction per issue) on
variance-reduced channels

  g2 = 2(t2 - mu2),          g3 = t(t2-a3),      g4 = (t2-a3)(t2-a4),
  g5 = 2*g3*(t2-a5),         g6 = 2*g4*(t2-a6),  g7 = g5*(t2-a7)

(t2 = tanh^2; factored orthogonal-ish polynomials whose rms is 3-10x
smaller than the raw powers, so fp8's 3.6% relative rounding hits much
smaller values; the 2x scales keep the channels out of fp8 subnormals
and divide out of the weights). The host exactly re-expresses the power
basis in these channels (triangular polynomial solve) and folds the
transform into the weights; constants fold into the k=0 bias.

Channels are built with bf16 DVE ops (fast 16-bit path, one mul per
channel) and cast to fp8 on the Act engine in k-major order matched to
the PE's PSUM-accumulation rounds. All 7 k-terms of an output tile
accumulate into a single PSUM bank (8 banks = 8 j-tiles in flight per
batch-half); DVE flushes each bank once to SBUF. Input DMAs are spread
over the sync/vector/gpsimd rings so x and the k-ordered weights land
just ahead of their consumers.
"""

import numpy as np
import ml_dtypes
from contextlib import ExitStack

B, IN, OUT, K = 8192, 1024, 1024, 8
NCORES = 8
BLOC = B // NCORES          # 1024 batch rows per core
NI = IN // 128              # 8 i-tiles
NJ = OUT // 128              # 8 j-tiles
NH = BLOC // 512            # 2 batch halves (PSUM bank = 512 fp32)

# Sequential weighted-least-squares roots of the factored channels under
# the empirical distribution of t = tanh(x), x ~ N(0,1), and E[t^2].
A3 = 0.641655
A4 = 0.153814
A5 = 0.500317
A6 = 0.705566
A7 = 0.570340
MU2 = 0.39426075880007483
# fp8 cast scales (applied on device, divided out of W on the host).
SC = {2: 2.0, 3: 1.0, 4: 1.0, 5: 2.0, 6: 2.0, 7: 2.0}

_NC_CACHE = {}


def _build_nc():
    import concourse.bacc as bacc
    import concourse.mybir as mybir
    import concourse.tile as tile

    dt = mybir.dt
    AF = mybir.ActivationFunctionType
    ALU = mybir.AluOpType
    DR = mybir.MatmulPerfMode.DoubleRow
    f32 = dt.float32
    bf16 = dt.bfloat16
    f8 = dt.float8e4

    nc = bacc.Bacc("TRN2", target_bir_lowering=False, debug=False)

    xt_d = nc.dram_tensor("xt", [IN, BLOC], bf16, kind="ExternalInput").ap()
    w1_d = nc.dram_tensor("w1", [128, NI, OUT], bf16, kind="ExternalInput").ap()
    wg_d = {k: nc.dram_tensor(f"wg{k}", [128, NI, OUT], f8,
                              kind="ExternalInput").ap() for k in range(2, K)}
    rng_d = nc.dram_tensor("rng", [1, 1], f32, kind="ExternalInput").ap()
    out_d = nc.dram_tensor("outT", [OUT, BLOC], f32, kind="ExternalOutput").ap()

    with tile.TileContext(nc) as tc, ExitStack() as ctx:
        sb = ctx.enter_context(tc.tile_pool(name="sb", bufs=1))
        xp = ctx.enter_context(tc.tile_pool(name="xp", bufs=3))
        sp = ctx.enter_context(tc.tile_pool(name="sp", bufs=2))
        wp = ctx.enter_context(tc.tile_pool(name="wp", bufs=9))
        op = ctx.enter_context(tc.tile_pool(name="op", bufs=4))
        pp = ctx.enter_context(tc.tile_pool(name="pp", bufs=8, space="PSUM"))

        # xs for h=1 as one persistent tile: a single 2MB DMA on the
        # vector ring overlaps the fine-grained h=0 stream on sync.
        xsh1 = sb.tile([128, NI, 512], bf16, tag="xsh1")
        r_col = sb.tile([128, 1], f32, tag="rcol")
        nc.sync.dma_start(r_col[:], rng_d.to_broadcast((128, 1)))

        # Persistent channel tensors, [128, i-tile, batch]
        b1 = sb.tile([128, NI, BLOC], bf16, tag="b1")
        g = {k: sb.tile([128, NI, BLOC], f8, tag=f"g{k}", name=f"g{k}")
             for k in range(2, K)}
        # Persistent weights
        w1s = sb.tile([128, NI, OUT], bf16, tag="w1s")
        wgs = {k: sb.tile([128, NI, OUT], f8, tag=f"wg{k}s", name=f"wg{k}s")
               for k in range(2, K)}

        ones = sb.tile([128, 512], bf16, tag="ones")
        nc.vector.memset(ones[:], 1.0)
        onesf = sb.tile([128, 1], f32, tag="onesf")
        nc.vector.memset(onesf[:], 1.0)

        # Preload the ACT tanh table before any real data arrives.
        warm = sb.tile([128, 1], f32, tag="warm")
        nc.scalar.activation(warm[:], onesf[:], AF.Tanh)

        # DMA plan. The three dynamic rings share the 16 DMA engines
        # (~400GB/s aggregate, saturated during startup), so the GLOBAL
        # trigger order must follow the PE's need order: x-h0/w1 first,
        # then wg2 (PE rounds run k1a, k2, k1b, k3..k7), then the rest.
        # gpsimd (slow SWDGE) ring: wg2 halves first, then wg5, wg7.
        nc.gpsimd.dma_start(wgs[2][:, 0:4, :], wg_d[2][:, 0:4, :])
        nc.gpsimd.dma_start(wgs[2][:, 4:NI, :], wg_d[2][:, 4:NI, :])
        nc.gpsimd.dma_start(wgs[5][:], wg_d[5][:])
        nc.gpsimd.dma_start(wgs[7][:], wg_d[7][:])

        # Tiny warmup matmuls ramp the PE clock without delaying k=1.
        wps = pp.tile([128, 512], f32, tag="ps")
        for wv in range(12):
            nc.tensor.matmul(wps[:, 0:128], ones[:, 0:128], ones[:, 0:128],
                             start=(wv == 0), stop=(wv == 11))

        # sync ring: even h=0 x chunks first (x has global priority:
        # everything depends on it), then w1 late half, xsh1, wg6.
        xss = {}
        for it in range(NI):
            xss[it] = xp.tile([128, 512], bf16, tag="xs", name=f"xs{it}")
        def xdma(eng, it):
            eng.dma_start(xss[it][:], xt_d[it * 128:(it + 1) * 128, 0:512])
        # sync ring: even x / even w1 interleaved in consumption order,
        # then wg3, xsh1 bulk, wg6.
        xdma(nc.sync, 0)
        xdma(nc.sync, 2)
        nc.sync.dma_start(w1s[:, 0, :], w1_d[:, 0, :])
        nc.sync.dma_start(w1s[:, 2, :], w1_d[:, 2, :])
        xdma(nc.sync, 4)
        xdma(nc.sync, 6)
        nc.sync.dma_start(w1s[:, 4, :], w1_d[:, 4, :])
        nc.sync.dma_start(w1s[:, 6, :], w1_d[:, 6, :])
        nc.sync.dma_start(wgs[3][:], wg_d[3][:])
        nc.sync.dma_start(xsh1[:], xt_d[:, 512:1024].rearrange(
            "(i p) b -> p i b", p=128))
        nc.sync.dma_start(wgs[6][:], wg_d[6][:])

        # Act ring: odd x / odd w1 interleaved, then wg4.
        xdma(nc.scalar, 1)
        xdma(nc.scalar, 3)
        nc.scalar.dma_start(w1s[:, 1, :], w1_d[:, 1, :])
        nc.scalar.dma_start(w1s[:, 3, :], w1_d[:, 3, :])
        xdma(nc.scalar, 5)
        xdma(nc.scalar, 7)
        nc.scalar.dma_start(w1s[:, 5, :], w1_d[:, 5, :])
        nc.scalar.dma_start(w1s[:, 7, :], w1_d[:, 7, :])
        nc.scalar.dma_start(wgs[4][:], wg_d[4][:])
        for it in range(NI):
            nc.scalar.activation(b1[:, it, 0:512], xss[it][:], AF.Tanh,
                                 scale=r_col[:, 0:1])


        b2s = {}

        def emit_dve_a(h):
            # pass A, level-major: all b2 muls first (g2 casts unblock
            # right behind the tanh stream), then v3/w3, then v4/w4.
            hs = slice(h * 512, (h + 1) * 512)
            tiles = []
            for it in range(NI):
                b2 = wp.tile([128, 512], bf16, tag="b2", name=f"b2_{h}_{it}")
                v3 = wp.tile([128, 512], bf16, tag="v3", name=f"v3_{h}_{it}")
                v4 = sp.tile([128, 512], bf16, tag="v4", name=f"v4_{h}_{it}",
                             bufs=4)
                w3 = wp.tile([128, 512], bf16, tag="w3", name=f"w3_{h}_{it}")
                w4 = wp.tile([128, 512], bf16, tag="w4", name=f"w4_{h}_{it}")
                b2s[(h, it)] = b2
                tiles.append((v3, v4, w3, w4))
            for it in range(NI):
                nc.vector.tensor_mul(b2s[(h, it)][:], b1[:, it, hs],
                                     b1[:, it, hs])
            for it in range(NI):
                v3, v4, w3, w4 = tiles[it]
                nc.vector.tensor_scalar_add(v3[:], b2s[(h, it)][:],
                                            float(-A3))
                nc.vector.tensor_mul(w3[:], v3[:], b1[:, it, hs])
            for it in range(NI):
                v3, v4, w3, w4 = tiles[it]
                nc.vector.tensor_scalar_add(v4[:], b2s[(h, it)][:],
                                            float(-A4))
                nc.vector.tensor_mul(w4[:], v3[:], v4[:])
            return [(w3, w4) for (v3, v4, w3, w4) in tiles]

        def emit_dve_b(h, ab):
            # pass B: w5 = 2*w3*(t2-a5), w6 = 2*w4*(t2-a6), w7 = w5*(t2-a7)
            out = []
            for it in range(NI):
                w3, w4 = ab[it]
                b2 = b2s[(h, it)]
                v5 = sp.tile([128, 512], bf16, tag="v5")
                v6 = sp.tile([128, 512], bf16, tag="v6")
                v7 = sp.tile([128, 512], bf16, tag="v7")
                w5 = sp.tile([128, 512], bf16, tag="w5")
                w6 = sp.tile([128, 512], bf16, tag="w6")
                w7 = sp.tile([128, 512], bf16, tag="w7")
                out.append((w5, w6, w7))
                nc.vector.tensor_scalar(v5[:], b2[:], float(-A5), 2.0,
                                        op0=ALU.add, op1=ALU.mult)
                nc.vector.tensor_mul(w5[:], w3[:], v5[:])
                nc.vector.tensor_scalar(v6[:], b2[:], float(-A6), 2.0,
                                        op0=ALU.add, op1=ALU.mult)
                nc.vector.tensor_mul(w6[:], w4[:], v6[:])
                nc.vector.tensor_scalar_add(v7[:], b2[:], float(-A7))
                nc.vector.tensor_mul(w7[:], w5[:], v7[:])
            return out

        def emit_casts_a(h, ab):
            # k-major so each PE round's channel completes as early as
            # possible; g5..g7 casts are deferred to emit_casts_b.
            hs = slice(h * 512, (h + 1) * 512)
            for it in range(NI):
                nc.scalar.activation(g[2][:, it, hs], b2s[(h, it)][:],
                                     AF.Copy, scale=float(SC[2]),
                                     bias=float(-SC[2] * MU2))
            for it in range(NI):
                nc.scalar.activation(g[3][:, it, hs], ab[it][0][:], AF.Copy)
            for it in range(NI):
                nc.scalar.activation(g[4][:, it, hs], ab[it][1][:], AF.Copy)

        def emit_casts_b(h, wb):
            hs = slice(h * 512, (h + 1) * 512)
            for m in range(3):
                for it in range(NI):
                    nc.scalar.activation(g[5 + m][:, it, hs], wb[it][m][:],
                                         AF.Copy)

        def emit_pe(h, pss):
            hs = slice(h * 512, (h + 1) * 512)

            def k1_half(iis):
                for ii in iis:
                    for j in range(NJ):
                        nc.tensor.matmul(
                            pss[j][:], w1s[:, ii, j * 128:(j + 1) * 128],
                            b1[:, ii, hs], start=(ii == 0), stop=False)

            def dr_round(k, ips):
                for ip in ips:
                    for j in range(NJ):
                        nc.tensor.matmul(
                            pss[j][:],
                            wgs[k][:, 2 * ip:2 * ip + 2,
                                   j * 128:(j + 1) * 128],
                            g[k][:, 2 * ip:2 * ip + 2, hs],
                            start=False,
                            stop=(k == K - 1 and ip == NI // 2 - 1),
                            perf_mode=DR)

            # h=0: fine-grained interleave of k1/k2 sub-rounds so early
            # PE consumption never outruns the saturated DMA pool;
            # h=1: natural order (everything is resident).
            if h == 0:
                k1_half(range(0, 2))
                k1_half(range(2, 4))
                dr_round(2, range(2))
                k1_half(range(4, 6))
                dr_round(2, range(2, 4))
                k1_half(range(6, NI))
            else:
                k1_half(range(NI))
                dr_round(2, range(4))
            for k in range(3, K):
                dr_round(k, range(4))

        def emit_flush(h, pss, engines):
            hs = slice(h * 512, (h + 1) * 512)
            for j in range(NJ):
                ot = op.tile([128, 512], f32, tag="ot")
                nc.vector.tensor_copy(ot[:], pss[j][:])
                engines[j % len(engines)].dma_start(
                    out_d[j * 128:(j + 1) * 128, hs], ot[:])

        pss = {h: [pp.tile([128, 512], f32, tag="ps", name=f"ps{h}_{j}")
                   for j in range(NJ)] for h in range(NH)}

        ab0 = emit_dve_a(0)
        emit_casts_a(0, ab0)
        wb0 = emit_dve_b(0, ab0)
        # h=1 tanh sits between the h0-A and h0-B cast groups: its xs
        # lands by ~17us and nothing on the PE needs it before ~70us.
        for it in range(NI):
            nc.scalar.activation(b1[:, it, 512:1024], xsh1[:, it, :],
                                 AF.Tanh, scale=r_col[:, 0:1])
        emit_casts_b(0, wb0)
        emit_pe(0, pss[0])
        ab1 = emit_dve_a(1)
        emit_casts_a(1, ab1)
        wb1 = emit_dve_b(1, ab1)
        emit_casts_b(1, wb1)
        emit_flush(0, pss[0], [nc.gpsimd])
        emit_pe(1, pss[1])
        emit_flush(1, pss[1], [nc.gpsimd, nc.sync])

    nc.compile()
    return nc


def _get_nc():
    if "nc" not in _NC_CACHE:
        _NC_CACHE["nc"] = _build_nc()
    return _NC_CACHE["nc"]


def _channel_polys():
    """Power-basis coefficients of the 7 channels, and the inverse map."""
    import numpy.polynomial.polynomial as P

    def pm(*ps):
        r = np.array([1.0])
        for p in ps:
            r = P.polymul(r, p)
        return r

    q = {k: np.array([-a, 0.0, 1.0]) for k, a in
         ((3, A3), (4, A4), (5, A5), (6, A6), (7, A7))}
    t = np.array([0.0, 1.0])
    CH = {1: t, 2: np.array([0.0, 0.0, 1.0]),
          3: pm(t, q[3]), 4: pm(q[3], q[4]), 5: pm(t, q[3], q[5]),
          6: pm(q[3], q[4], q[6]), 7: pm(t, q[3], q[5], q[7])}
    C = np.zeros((7, 8))
    for m in range(1, 8):
        cc = CH[m]
        C[m - 1, :len(cc)] = cc
    M = C[:, 1:8]                       # channel_m = consts + M @ powers
    consts = C[:, 0]
    Binv = np.linalg.inv(M)             # powers = Binv @ (channels - consts)
    return Binv, consts


def _pack_w(w, dtype):
    # [OUT, IN] -> [128, NI, OUT]
    a = np.ascontiguousarray(w.T.reshape(NI, 128, OUT).transpose(1, 0, 2))
    return a.astype(dtype)


def _make_in_maps(x, tanh_range, coef):
    x = np.asarray(x, dtype=np.float32)
    coef = np.asarray(coef, dtype=np.float64)
    W = {k: coef[:, :, k] for k in range(K)}
    Binv, consts = _channel_polys()
    Wp = {m: sum(W[k] * Binv[k - 1, m - 1] for k in range(1, 8))
          for m in range(1, 8)}
    s = W[0].sum(axis=1)
    for k in range(1, 8):
        cst = sum(Binv[k - 1, m - 1] * consts[m - 1] for m in range(1, 8))
        s -= cst * W[k].sum(axis=1)
    # channel 2 is fed as SC2*(t2 - MU2): fold the mean term into s.
    s += MU2 * Wp[2].sum(axis=1)
    bf = ml_dtypes.bfloat16
    f8 = ml_dtypes.float8_e4m3
    shared = {"w1": _pack_w(Wp[1], bf),
              "rng": np.asarray(tanh_range, np.float32).reshape(1, 1)}
    for k in range(2, K):
        shared[f"wg{k}"] = _pack_w(Wp[k] / SC[k], f8)
    in_maps = []
    xbf = x.astype(ml_dtypes.bfloat16)
    for c in range(NCORES):
        xt = np.ascontiguousarray(xbf[c * BLOC:(c + 1) * BLOC, :].T)
        in_maps.append({"xt": xt, **shared})
    return in_maps, s.astype(np.float32)


def _ensure_ntff_hook():
    """Register the axon NTFF profile hook if the image's antenv lacks it."""
    import sys
    import types
    try:
        from antenv.axon_hooks import get_axon_ntff_profile_hook  # noqa: F401
        return
    except ImportError:
        pass
    try:
        from trn_agent_boot.trn_boot import _ntff_profile_via_ctypes
        hook = _ntff_profile_via_ctypes("/opt/axon/libaxon_pjrt.so")
    except Exception:
        hook = None
    mod = types.ModuleType("antenv.axon_hooks")
    state = {"hook": hook}
    mod.set_axon_ntff_profile_hook = lambda h: state.__setitem__("hook", h)
    mod.get_axon_ntff_profile_hook = lambda: state["hook"]
    sys.modules["antenv.axon_hooks"] = mod
    import antenv
    antenv.axon_hooks = mod


def _run(x, tanh_range, coef, trace=False):
    from concourse.bass_utils import run_bass_kernel_spmd

    if trace:
        _ensure_ntff_hook()

    nc = _get_nc()
    in_maps, s = _make_in_maps(x, tanh_range, coef)
    res = run_bass_kernel_spmd(nc, in_maps, core_ids=list(range(NCORES)),
                               trace=trace)
    out = np.empty((B, OUT), dtype=np.float32)
    for c in range(NCORES):
        out[c * BLOC:(c + 1) * BLOC, :] = res.results[c]["outT"].T
    out += s[None, :]
    return out, res


def kernel(x, tanh_range, coef):
    out, _ = _run(x, tanh_range, coef, trace=False)
    return out


# revision 20
# speedup vs baseline: 1.0863x; 1.0655x over previous
"""Trainium2 Bass kernel for CustomTaylorLayer.

Computes out[b, j] = sum_{i,k} coef[j, i, k] * tanh(x[b, i] * r)^k
for x:[8192,1024], coef:[1024,1024,8], r scalar.

Strategy: data-parallel over the batch across 8 NeuronCores (1024 rows
per core). The k=0 term is an exact host-side column sum added after the
gather. k=1 runs as bf16 matmuls on b1=tanh. k=2..7 run as fp8(e4m3)
DoubleRow matmuls (2 weights/PE cell, 256-wide contraction per issue) on
variance-reduced channels

  g2 = 2(t2 - mu2),          g3 = t(t2-a3),      g4 = (t2-a3)(t2-a4),
  g5 = 2*g3*(t2-a5),         g6 = 2*g4*(t2-a6),  g7 = g5*(t2-a7)

(t2 = tanh^2; factored orthogonal-ish polynomials whose rms is 3-10x
smaller than the raw powers, so fp8's 3.6% relative rounding hits much
smaller values; the 2x scales keep the channels out of fp8 subnormals
and divide out of the weights). The host exactly re-expresses the power
basis in these channels (triangular polynomial solve) and folds the
transform into the weights; constants fold into the k=0 bias.

Channels are built with bf16 DVE ops (fast 16-bit path, one mul per
channel) and cast to fp8 on the Act engine in k-major order matched to
the PE's PSUM-accumulation rounds. All 7 k-terms of an output tile
accumulate into a single PSUM bank (8 banks = 8 j-tiles in flight per
batch-half); DVE flushes each bank once to SBUF. Input DMAs are spread
over the sync/vector/gpsimd rings so x and the k-ordered weights land
just ahead of their consumers.
"""

import numpy as np
import ml_dtypes
from contextlib import ExitStack

B, IN, OUT, K = 8192, 1024, 1024, 8
NCORES = 8
BLOC = B // NCORES          # 1024 batch rows per core
NI = IN // 128              # 8 i-tiles
NJ = OUT // 128              # 8 j-tiles
NH = BLOC // 512            # 2 batch halves (PSUM bank = 512 fp32)

# Sequential weighted-least-squares roots of the factored channels under
# the empirical distribution of t = tanh(x), x ~ N(0,1), and E[t^2].
A3 = 0.641655
A4 = 0.153814
A5 = 0.500317
A6 = 0.705566
A7 = 0.570340
MU2 = 0.39426075880007483
# fp8 cast scales (applied on device, divided out of W on the host).
SC = {2: 2.0, 3: 1.0, 4: 1.0, 5: 2.0, 6: 2.0, 7: 2.0}

_NC_CACHE = {}


def _build_nc():
    import concourse.bacc as bacc
    import concourse.mybir as mybir
    import concourse.tile as tile

    dt = mybir.dt
    AF = mybir.ActivationFunctionType
    ALU = mybir.AluOpType
    DR = mybir.MatmulPerfMode.DoubleRow
    f32 = dt.float32
    bf16 = dt.bfloat16
    f8 = dt.float8e4

    nc = bacc.Bacc("TRN2", target_bir_lowering=False, debug=False)

    xt_d = nc.dram_tensor("xt", [IN, BLOC], bf16, kind="ExternalInput").ap()
    w1_d = nc.dram_tensor("w1", [128, NI, OUT], bf16, kind="ExternalInput").ap()
    wg_d = {k: nc.dram_tensor(f"wg{k}", [128, NI, OUT], f8,
                              kind="ExternalInput").ap() for k in range(2, K)}
    rng_d = nc.dram_tensor("rng", [1, 1], f32, kind="ExternalInput").ap()
    out_d = nc.dram_tensor("outT", [OUT, BLOC], f32, kind="ExternalOutput").ap()

    with tile.TileContext(nc) as tc, ExitStack() as ctx:
        sb = ctx.enter_context(tc.tile_pool(name="sb", bufs=1))
        xp = ctx.enter_context(tc.tile_pool(name="xp", bufs=3))
        sp = ctx.enter_context(tc.tile_pool(name="sp", bufs=2))
        wp = ctx.enter_context(tc.tile_pool(name="wp", bufs=9))
        op = ctx.enter_context(tc.tile_pool(name="op", bufs=4))
        pp = ctx.enter_context(tc.tile_pool(name="pp", bufs=8, space="PSUM"))

        # xs for h=1 as one persistent tile: a single 2MB DMA on the
        # vector ring overlaps the fine-grained h=0 stream on sync.
        xsh1 = sb.tile([128, NI, 512], bf16, tag="xsh1")
        r_col = sb.tile([128, 1], f32, tag="rcol")
        nc.sync.dma_start(r_col[:], rng_d.to_broadcast((128, 1)))

        # Persistent channel tensors, [128, i-tile, batch]
        b1 = sb.tile([128, NI, BLOC], bf16, tag="b1")
        g = {k: sb.tile([128, NI, BLOC], f8, tag=f"g{k}", name=f"g{k}")
             for k in range(2, K)}
        # Persistent weights
        w1s = sb.tile([128, NI, OUT], bf16, tag="w1s")
        wgs = {k: sb.tile([128, NI, OUT], f8, tag=f"wg{k}s", name=f"wg{k}s")
               for k in range(2, K)}

        ones = sb.tile([128, 512], bf16, tag="ones")
        nc.vector.memset(ones[:], 1.0)
        onesf = sb.tile([128, 1], f32, tag="onesf")
        nc.vector.memset(onesf[:], 1.0)

        # Preload the ACT tanh table before any real data arrives.
        warm = sb.tile([128, 1], f32, tag="warm")
        nc.scalar.activation(warm[:], onesf[:], AF.Tanh)

        # DMA plan. Dynamic-ring DMAs pay ~1.5-2us fixed overhead each
        # and the three rings share the 16-engine pool (~400GB/s), so
        # transfers are few/large and ordered by PE need-time per ring.
        # gpsimd (slow SWDGE) ring: wg2 halves, wg3, wg5, wg7.
        nc.gpsimd.dma_start(wgs[2][:, 0:4, :], wg_d[2][:, 0:4, :])
        nc.gpsimd.dma_start(wgs[2][:, 4:NI, :], wg_d[2][:, 4:NI, :])
        nc.gpsimd.dma_start(wgs[3][:], wg_d[3][:])
        nc.gpsimd.dma_start(wgs[5][:], wg_d[5][:])
        nc.gpsimd.dma_start(wgs[7][:], wg_d[7][:])

        # Tiny warmup matmuls ramp the PE clock without delaying k=1.
        wps = pp.tile([128, 512], f32, tag="ps")
        for wv in range(12):
            nc.tensor.matmul(wps[:, 0:128], ones[:, 0:128], ones[:, 0:128],
                             start=(wv == 0), stop=(wv == 11))

        # sync ring: h=0 x in two bulk chunks, then w1 late half, wg4,
        # xsh1 bulk, wg6.
        xsh0 = sb.tile([128, NI, 512], bf16, tag="xsh0")
        nc.sync.dma_start(xsh0[:, 0:4, :], xt_d[:, 0:512].rearrange(
            "(i p) b -> p i b", p=128)[:, 0:4, :])
        nc.sync.dma_start(xsh0[:, 4:NI, :], xt_d[:, 0:512].rearrange(
            "(i p) b -> p i b", p=128)[:, 4:NI, :])
        for ii in range(4, NI):
            nc.sync.dma_start(w1s[:, ii, :], w1_d[:, ii, :])
        nc.sync.dma_start(wgs[4][:], wg_d[4][:])
        nc.sync.dma_start(xsh1[:], xt_d[:, 512:1024].rearrange(
            "(i p) b -> p i b", p=128))
        nc.sync.dma_start(wgs[6][:], wg_d[6][:])

        # Act ring: the early half of w1 only, so the Act FIFO reaches
        # the tanh stream quickly.
        for ii in range(4):
            nc.scalar.dma_start(w1s[:, ii, :], w1_d[:, ii, :])
        for it in range(NI):
            nc.scalar.activation(b1[:, it, 0:512], xsh0[:, it, :], AF.Tanh,
                                 scale=r_col[:, 0:1])

        b2s = {}

        def emit_dve_a(h):
            # pass A, level-major: all b2 muls first (g2 casts unblock
            # right behind the tanh stream), then v3/w3, then v4/w4.
            hs = slice(h * 512, (h + 1) * 512)
            tiles = []
            for it in range(NI):
                b2 = wp.tile([128, 512], bf16, tag="b2", name=f"b2_{h}_{it}")
                v3 = wp.tile([128, 512], bf16, tag="v3", name=f"v3_{h}_{it}")
                v4 = sp.tile([128, 512], bf16, tag="v4", name=f"v4_{h}_{it}",
                             bufs=4)
                w3 = wp.tile([128, 512], bf16, tag="w3", name=f"w3_{h}_{it}")
                w4 = wp.tile([128, 512], bf16, tag="w4", name=f"w4_{h}_{it}")
                b2s[(h, it)] = b2
                tiles.append((v3, v4, w3, w4))
            for it in range(NI):
                nc.vector.tensor_mul(b2s[(h, it)][:], b1[:, it, hs],
                                     b1[:, it, hs])
            for it in range(NI):
                v3, v4, w3, w4 = tiles[it]
                nc.vector.tensor_scalar_add(v3[:], b2s[(h, it)][:],
                                            float(-A3))
                nc.vector.tensor_mul(w3[:], v3[:], b1[:, it, hs])
            for it in range(NI):
                v3, v4, w3, w4 = tiles[it]
                nc.vector.tensor_scalar_add(v4[:], b2s[(h, it)][:],
                                            float(-A4))
                nc.vector.tensor_mul(w4[:], v3[:], v4[:])
            return [(w3, w4) for (v3, v4, w3, w4) in tiles]

        def emit_dve_b(h, ab):
            # pass B: w5 = 2*w3*(t2-a5), w6 = 2*w4*(t2-a6), w7 = w5*(t2-a7)
            out = []
            for it in range(NI):
                w3, w4 = ab[it]
                b2 = b2s[(h, it)]
                v5 = sp.tile([128, 512], bf16, tag="v5")
                v6 = sp.tile([128, 512], bf16, tag="v6")
                v7 = sp.tile([128, 512], bf16, tag="v7")
                w5 = sp.tile([128, 512], bf16, tag="w5")
                w6 = sp.tile([128, 512], bf16, tag="w6")
                w7 = sp.tile([128, 512], bf16, tag="w7")
                out.append((w5, w6, w7))
                nc.vector.tensor_scalar(v5[:], b2[:], float(-A5), 2.0,
                                        op0=ALU.add, op1=ALU.mult)
                nc.vector.tensor_mul(w5[:], w3[:], v5[:])
                nc.vector.tensor_scalar(v6[:], b2[:], float(-A6), 2.0,
                                        op0=ALU.add, op1=ALU.mult)
                nc.vector.tensor_mul(w6[:], w4[:], v6[:])
                nc.vector.tensor_scalar_add(v7[:], b2[:], float(-A7))
                nc.vector.tensor_mul(w7[:], w5[:], v7[:])
            return out

        def emit_casts_a(h, ab):
            # k-major so each PE round's channel completes as early as
            # possible; g5..g7 casts are deferred to emit_casts_b.
            hs = slice(h * 512, (h + 1) * 512)
            for it in range(NI):
                nc.scalar.activation(g[2][:, it, hs], b2s[(h, it)][:],
                                     AF.Copy, scale=float(SC[2]),
                                     bias=float(-SC[2] * MU2))
            for it in range(NI):
                nc.scalar.activation(g[3][:, it, hs], ab[it][0][:], AF.Copy)
            for it in range(NI):
                nc.scalar.activation(g[4][:, it, hs], ab[it][1][:], AF.Copy)

        def emit_casts_b(h, wb):
            hs = slice(h * 512, (h + 1) * 512)
            for m in range(3):
                for it in range(NI):
                    nc.scalar.activation(g[5 + m][:, it, hs], wb[it][m][:],
                                         AF.Copy)

        def emit_pe(h, pss):
            hs = slice(h * 512, (h + 1) * 512)

            def k1_half(iis):
                for ii in iis:
                    for j in range(NJ):
                        nc.tensor.matmul(
                            pss[j][:], w1s[:, ii, j * 128:(j + 1) * 128],
                            b1[:, ii, hs], start=(ii == 0), stop=False)

            def dr_round(k, ips):
                for ip in ips:
                    for j in range(NJ):
                        nc.tensor.matmul(
                            pss[j][:],
                            wgs[k][:, 2 * ip:2 * ip + 2,
                                   j * 128:(j + 1) * 128],
                            g[k][:, 2 * ip:2 * ip + 2, hs],
                            start=False,
                            stop=(k == K - 1 and ip == NI // 2 - 1),
                            perf_mode=DR)

            # h=0: fine-grained interleave of k1/k2 sub-rounds so early
            # PE consumption never outruns the saturated DMA pool;
            # h=1: natural order (everything is resident).
            if h == 0:
                k1_half(range(0, 2))
                k1_half(range(2, 4))
                dr_round(2, range(2))
                k1_half(range(4, 6))
                dr_round(2, range(2, 4))
                k1_half(range(6, NI))
            else:
                k1_half(range(NI))
                dr_round(2, range(4))
            for k in range(3, K):
                dr_round(k, range(4))

        def emit_flush(h, pss, engines):
            hs = slice(h * 512, (h + 1) * 512)
            for j in range(NJ):
                ot = op.tile([128, 512], f32, tag="ot")
                nc.vector.tensor_copy(ot[:], pss[j][:])
                engines[j % len(engines)].dma_start(
                    out_d[j * 128:(j + 1) * 128, hs], ot[:])

        pss = {h: [pp.tile([128, 512], f32, tag="ps", name=f"ps{h}_{j}")
                   for j in range(NJ)] for h in range(NH)}

        ab0 = emit_dve_a(0)
        emit_casts_a(0, ab0)
        wb0 = emit_dve_b(0, ab0)
        # h=1 tanh sits between the h0-A and h0-B cast groups: its xs
        # lands by ~17us and nothing on the PE needs it before ~70us.
        for it in range(NI):
            nc.scalar.activation(b1[:, it, 512:1024], xsh1[:, it, :],
                                 AF.Tanh, scale=r_col[:, 0:1])
        emit_casts_b(0, wb0)
        emit_pe(0, pss[0])
        ab1 = emit_dve_a(1)
        emit_casts_a(1, ab1)
        wb1 = emit_dve_b(1, ab1)
        emit_casts_b(1, wb1)
        emit_flush(0, pss[0], [nc.gpsimd])
        emit_pe(1, pss[1])
        emit_flush(1, pss[1], [nc.gpsimd, nc.sync])

    nc.compile()
    return nc


def _get_nc():
    if "nc" not in _NC_CACHE:
        _NC_CACHE["nc"] = _build_nc()
    return _NC_CACHE["nc"]


def _channel_polys():
    """Power-basis coefficients of the 7 channels, and the inverse map."""
    import numpy.polynomial.polynomial as P

    def pm(*ps):
        r = np.array([1.0])
        for p in ps:
            r = P.polymul(r, p)
        return r

    q = {k: np.array([-a, 0.0, 1.0]) for k, a in
         ((3, A3), (4, A4), (5, A5), (6, A6), (7, A7))}
    t = np.array([0.0, 1.0])
    CH = {1: t, 2: np.array([0.0, 0.0, 1.0]),
          3: pm(t, q[3]), 4: pm(q[3], q[4]), 5: pm(t, q[3], q[5]),
          6: pm(q[3], q[4], q[6]), 7: pm(t, q[3], q[5], q[7])}
    C = np.zeros((7, 8))
    for m in range(1, 8):
        cc = CH[m]
        C[m - 1, :len(cc)] = cc
    M = C[:, 1:8]                       # channel_m = consts + M @ powers
    consts = C[:, 0]
    Binv = np.linalg.inv(M)             # powers = Binv @ (channels - consts)
    return Binv, consts


def _pack_w(w, dtype):
    # [OUT, IN] -> [128, NI, OUT]
    a = np.ascontiguousarray(w.T.reshape(NI, 128, OUT).transpose(1, 0, 2))
    return a.astype(dtype)


def _make_in_maps(x, tanh_range, coef):
    x = np.asarray(x, dtype=np.float32)
    coef = np.asarray(coef, dtype=np.float64)
    W = {k: coef[:, :, k] for k in range(K)}
    Binv, consts = _channel_polys()
    Wp = {m: sum(W[k] * Binv[k - 1, m - 1] for k in range(1, 8))
          for m in range(1, 8)}
    s = W[0].sum(axis=1)
    for k in range(1, 8):
        cst = sum(Binv[k - 1, m - 1] * consts[m - 1] for m in range(1, 8))
        s -= cst * W[k].sum(axis=1)
    # channel 2 is fed as SC2*(t2 - MU2): fold the mean term into s.
    s += MU2 * Wp[2].sum(axis=1)
    bf = ml_dtypes.bfloat16
    f8 = ml_dtypes.float8_e4m3
    shared = {"w1": _pack_w(Wp[1], bf),
              "rng": np.asarray(tanh_range, np.float32).reshape(1, 1)}
    for k in range(2, K):
        shared[f"wg{k}"] = _pack_w(Wp[k] / SC[k], f8)
    in_maps = []
    xbf = x.astype(ml_dtypes.bfloat16)
    for c in range(NCORES):
        xt = np.ascontiguousarray(xbf[c * BLOC:(c + 1) * BLOC, :].T)
        in_maps.append({"xt": xt, **shared})
    return in_maps, s.astype(np.float32)


def _ensure_ntff_hook():
    """Register the axon NTFF profile hook if the image's antenv lacks it."""
    import sys
    import types
    try:
        from antenv.axon_hooks import get_axon_ntff_profile_hook  # noqa: F401
        return
    except ImportError:
        pass
    try:
        from trn_agent_boot.trn_boot import _ntff_profile_via_ctypes
        hook = _ntff_profile_via_ctypes("/opt/axon/libaxon_pjrt.so")
    except Exception:
        hook = None
    mod = types.ModuleType("antenv.axon_hooks")
    state = {"hook": hook}
    mod.set_axon_ntff_profile_hook = lambda h: state.__setitem__("hook", h)
    mod.get_axon_ntff_profile_hook = lambda: state["hook"]
    sys.modules["antenv.axon_hooks"] = mod
    import antenv
    antenv.axon_hooks = mod


def _run(x, tanh_range, coef, trace=False):
    from concourse.bass_utils import run_bass_kernel_spmd

    if trace:
        _ensure_ntff_hook()

    nc = _get_nc()
    in_maps, s = _make_in_maps(x, tanh_range, coef)
    res = run_bass_kernel_spmd(nc, in_maps, core_ids=list(range(NCORES)),
                               trace=trace)
    out = np.empty((B, OUT), dtype=np.float32)
    for c in range(NCORES):
        out[c * BLOC:(c + 1) * BLOC, :] = res.results[c]["outT"].T
    out += s[None, :]
    return out, res


def kernel(x, tanh_range, coef):
    out, _ = _run(x, tanh_range, coef, trace=False)
    return out
